# revision 36
# baseline (speedup 1.0000x reference)
"""Causal multi-head self-attention with RoPE on 8 NeuronCores.

Sharding: batch (4) x head-halves (2) -> 8 cores. Core c handles batch c//2,
heads [8*(c%2), 8*(c%2)+8). Pipelined schedule: QKV projection, attention
i-chunks and output projection are interleaved at tile granularity so the
Scalar engine (softmax exp) and PE (matmuls) overlap across the whole kernel.
The pair of cores sharing a batch exchanges normalized attention outputs with
a per-chunk AllGather; each core then projects its own half of the output
columns with the full 1024-dim contraction (no AllReduce, half the traffic).
"""

import numpy as np
import ml_dtypes

import concourse.bacc as bacc
import concourse.bass as bass
import concourse.mybir as mybir
from concourse.tile import TileContext
from concourse.bass_utils import run_bass_kernel_spmd

B, S, D, H = 4, 2048, 1024, 16
HL = 8          # heads per core
DK = 64         # head dim
NCORES = 8
DT = D // 128   # 8 d-tiles (contraction tiles)
OT = HL * DK // 128   # 4 o-tiles for Q^T/K^T ([128, S] each, 2 heads per tile)
ST = S // 128   # 16 s-tiles
VW = DK + 1     # V columns per head incl. ones column

BF16 = mybir.dt.bfloat16
F32 = mybir.dt.float32
NEG = -1.0e9
EXP = mybir.ActivationFunctionType.Exp

_compiled = {}


def _build_nc():
    nc = bacc.Bacc("TRN2", target_bir_lowering=False, debug=False,
                   num_devices=NCORES)

    xT = nc.dram_tensor("xT", [D, S], BF16, kind="ExternalInput")
    wqT = nc.dram_tensor("wqT", [D, HL * DK], BF16, kind="ExternalInput")
    wkT = nc.dram_tensor("wkT", [D, HL * DK], BF16, kind="ExternalInput")
    wvT = nc.dram_tensor("wvT", [D, HL * DK], BF16, kind="ExternalInput")
    woT = nc.dram_tensor("woT", [D, D // 2], BF16, kind="ExternalInput")
    cosT = nc.dram_tensor("cosT", [128, S], BF16, kind="ExternalInput")
    sinT = nc.dram_tensor("sinT", [128, S], BF16, kind="ExternalInput")
    swapT = nc.dram_tensor("swapT", [128, 128], BF16, kind="ExternalInput")
    maskT = nc.dram_tensor("maskT", [128, 128], BF16, kind="ExternalInput")
    y = nc.dram_tensor("y", [S, D // 2], BF16, kind="ExternalOutput")

    groups = [[0, 1], [2, 3], [4, 5], [6, 7]]

    with TileContext(nc) as tc:
        with (
            tc.tile_pool(name="big", bufs=1) as big,
            tc.tile_pool(name="rawp", bufs=2) as rawp,
            tc.tile_pool(name="t1p", bufs=2) as t1p,
            tc.tile_pool(name="t2p", bufs=2) as t2p,
            tc.tile_pool(name="ptp", bufs=12) as ptp,
            tc.tile_pool(name="osbp", bufs=10) as osbp,
            tc.tile_pool(name="otp", bufs=8) as otp,
            tc.tile_pool(name="oap", bufs=8) as oap,
            tc.tile_pool(name="ychp", bufs=3) as ychp,
            tc.tile_pool(name="nrmp", bufs=2) as nrmp,
            tc.tile_pool(name="stgp", bufs=3) as stgp,
            tc.tile_pool(name="repp", bufs=3) as repp,
            tc.tile_pool(name="ps_b", bufs=2, space="PSUM") as ps_b,
            tc.tile_pool(name="ps_o", bufs=4, space="PSUM") as ps_o,
            tc.tile_pool(name="dram", bufs=1, space="DRAM") as dram,
        ):
            # ---- input DMAs, split across queues so QKV can start early ----
            w_sb = {"q": [None] * DT, "k": [None] * DT, "v": [None] * DT}
            xT_sb = [None] * DT
            for k in range(DT):
                e1, e2 = (nc.sync, nc.scalar) if k % 2 == 0 else (nc.scalar, nc.sync)
                xt = big.tile([128, S], BF16, tag=f"xT{k}", name=f"xT{k}")
                e1.dma_start(out=xt[:, 0:1024], in_=xT[128 * k:128 * (k + 1),
                                                       0:1024])
                xT_sb[k] = xt
                wt = big.tile([128, HL * DK], BF16, tag=f"wq{k}", name=f"wq{k}")
                e2.dma_start(out=wt[:], in_=wqT[128 * k:128 * (k + 1), :])
                w_sb["q"][k] = wt
            cos_sb = big.tile([128, S], BF16, tag="cos")
            nc.scalar.dma_start(out=cos_sb[:], in_=cosT[:])
            sin_sb = big.tile([128, S], BF16, tag="sin")
            nc.scalar.dma_start(out=sin_sb[:], in_=sinT[:])
            swap_sb = big.tile([128, 128], BF16, tag="swap")
            nc.scalar.dma_start(out=swap_sb[:], in_=swapT[:])
            for k in range(DT):
                e1 = nc.sync if k % 2 == 0 else nc.scalar
                wt = big.tile([128, HL * DK], BF16, tag=f"wk{k}", name=f"wk{k}")
                e1.dma_start(out=wt[:], in_=wkT[128 * k:128 * (k + 1), :])
                w_sb["k"][k] = wt
            mask_sb = big.tile([128, 128], BF16, tag="mask")
            nc.scalar.dma_start(out=mask_sb[:], in_=maskT[:])
            for k in range(DT):
                e1 = nc.sync if k % 2 == 0 else nc.scalar
                e1.dma_start(out=xT_sb[k][:, 1024:2048],
                             in_=xT[128 * k:128 * (k + 1), 1024:2048])
            for k in range(DT):
                e1 = nc.sync if k % 2 == 0 else nc.scalar
                wt = big.tile([128, HL * DK], BF16, tag=f"wv{k}", name=f"wv{k}")
                e1.dma_start(out=wt[:], in_=wvT[128 * k:128 * (k + 1), :])
                w_sb["v"][k] = wt
            woT_sb = []
            for k in range(DT):
                e1 = nc.sync if k % 2 == 0 else nc.scalar
                wt = big.tile([128, D // 2], BF16, tag=f"wo{k}", name=f"wo{k}")
                e1.dma_start(out=wt[:], in_=woT[128 * k:128 * (k + 1), :])
                woT_sb.append(wt)

            vaug = []
            for st in range(ST):
                vt = big.tile([128, HL * VW], BF16, tag=f"va{st}", name=f"va{st}")
                ones = vt[:].rearrange("p (h d) -> p h d", d=VW)[:, :, DK:VW]
                nc.gpsimd.memset(ones, 1.0)
                vaug.append(vt)
            qrot = [big.tile([128, S], BF16, tag=f"qr{t}", name=f"qr{t}")
                    for t in range(OT)]
            krot = [big.tile([128, S], BF16, tag=f"kr{t}", name=f"kr{t}")
                    for t in range(OT)]

            oT = {}        # m -> [OT tiles [128,512]] my normalized o chunk
            oPeer = {}     # m -> [OT tiles [128,512]] peer half via AllGather
            osb_map = {}   # (m, h) -> osb tile
            sums_map = {}  # m -> sums8 tile
            # o exchange buffers, half-chunk granular: half hp of chunk m is
            # og_in rows [512m+256hp, +256) = heads 4hp..4hp+3 of this core,
            # cols = 512 i's of chunk m. og_out block [1024m+512hp, +512) =
            # [rank0's 256 rows; rank1's 256 rows].
            og_in = dram.tile([4 * 512, 512], BF16)
            og_out = dram.tile([4 * 1024, 512], BF16)

            qk_state = {}

            def qk_proj(h, qk, t):
                c0 = 1024 * h
                ps = ps_b.tile([128, 1024], F32, tag="psb", name="ps")
                for k in range(DT):
                    for cc in range(2):
                        nc.tensor.matmul(
                            ps[:, 512 * cc:512 * (cc + 1)],
                            lhsT=w_sb[qk][k][:, 128 * t:128 * (t + 1)],
                            rhs=xT_sb[k][:, c0 + 512 * cc:c0 + 512 * (cc + 1)],
                            start=(k == 0), stop=(k == DT - 1))
                raw = rawp.tile([128, 1024], BF16, tag="raw")
                if qk == "q":
                    nc.scalar.copy(out=raw[:], in_=ps[:])
                else:
                    nc.vector.tensor_copy(raw[:], ps[:])
                t1 = t1p.tile([128, 1024], BF16, tag="t1")
                nc.vector.tensor_mul(t1[:], raw[:], cos_sb[:, c0:c0 + 1024])
                qk_state[(h, qk, t)] = (raw, t1)

            def qk_swap(h, qk, t):
                c0 = 1024 * h
                raw, t1 = qk_state.pop((h, qk, t))
                ps2 = ps_b.tile([128, 1024], F32, tag="psb", name="ps2")
                for cc in range(2):
                    nc.tensor.matmul(
                        ps2[:, 512 * cc:512 * (cc + 1)], lhsT=swap_sb[:],
                        rhs=raw[:, 512 * cc:512 * (cc + 1)],
                        start=True, stop=True)
                t2 = t2p.tile([128, 1024], BF16, tag="t2")
                nc.vector.tensor_mul(t2[:], ps2[:], sin_sb[:, c0:c0 + 1024])
                dst = qrot[t] if qk == "q" else krot[t]
                nc.vector.tensor_add(dst[:, c0:c0 + 1024], t1[:], t2[:])

            def qk_unit(h, qk, t):
                qk_proj(h, qk, t)
                qk_swap(h, qk, t)

            def v_unit(st):
                ps = ps_b.tile([128, 1024], F32, tag="psb", name="psv")[:, 0:512]
                for k in range(DT):
                    nc.tensor.matmul(
                        ps[:], lhsT=xT_sb[k][:, 128 * st:128 * (st + 1)],
                        rhs=w_sb["v"][k][:], start=(k == 0), stop=(k == DT - 1))
                dst = vaug[st][:].rearrange("p (h d) -> p h d", d=VW)[:, :, 0:DK]
                src = ps[:].rearrange("p (h d) -> p h d", d=DK)
                nc.vector.tensor_copy(dst, src)

            def _attn_slot(m, tp, jb):
                # QK^T (row-tiled head pair) -> exp -> post-exp 0/1 mask
                i0 = 512 * m
                j0 = 128 * jb
                dlt = max(0, j0 - i0)
                s_ps = ps_b.tile([128, 1024], F32, tag="psb", name="sps")
                for half, po in ((0, 0), (1, DK)):
                    nc.tensor.matmul(
                        s_ps[:, 512 * half + dlt:512 * (half + 1)],
                        lhsT=krot[tp][po:po + DK, j0:j0 + 128],
                        rhs=qrot[tp][po:po + DK, i0 + dlt:i0 + 512],
                        start=True, stop=True)
                pT = ptp.tile([128, 1024], BF16, tag="pT")
                pT3 = pT[:].rearrange("p (b f) -> p b f", b=2)
                nc.scalar.activation(
                    pT3[:, :, dlt:512],
                    s_ps[:].rearrange("p (b f) -> p b f", b=2)[:, :, dlt:512],
                    EXP, scale=0.125)
                if j0 >= i0:
                    nc.vector.tensor_mul(
                        pT3[:, :, dlt:dlt + 128],
                        pT3[:, :, dlt:dlt + 128],
                        mask_sb[:].rearrange("p (b f) -> p b f", b=1)
                        .broadcast_to([128, 2, 128]))
                return pT

            def _attn_pv(m, tp, jb, pT, accs, start, stop):
                dlt = max(0, 128 * jb - 512 * m)
                for half in (0, 1):
                    nc.tensor.matmul(
                        accs[half][:, dlt:512],
                        lhsT=vaug[jb][:, VW * (2 * tp + half):
                                      VW * (2 * tp + half + 1)],
                        rhs=pT[:, 512 * half + dlt:512 * (half + 1)],
                        start=start, stop=stop,
                    )

            def _attn_drain(m, tp, accs, pair_base):
                osb_e = osbp.tile([VW, 512], BF16, tag="osb", name="osbe")
                nc.vector.tensor_copy(osb_e[:], accs[0][:])
                osb_o = osbp.tile([VW, 512], BF16, tag="osb", name="osbo")
                nc.vector.tensor_copy(osb_o[:], accs[1][:])
                hp = pair_base // 2
                if (m, hp) not in sums_map:
                    sums_map[(m, hp)] = nrmp.tile([4, 512], BF16, tag="sums4",
                                                  name=f"sums4_{m}_{hp}")
                sums4 = sums_map[(m, hp)]
                r = 2 * (tp - pair_base)
                nc.sync.dma_start(out=sums4[r:r + 1, :], in_=osb_e[DK:VW, :])
                nc.sync.dma_start(out=sums4[r + 1:r + 2, :],
                                  in_=osb_o[DK:VW, :])
                osb_map[(m, 2 * tp)] = osb_e
                osb_map[(m, 2 * tp + 1)] = osb_o

            def attn_pair(m, tps, fillers=()):
                # two head-pair tiles interleaved slot-by-slot so the PE queue
                # always has ready work; PV in groups of 4 j-blocks per tp
                i0 = 512 * m
                njb = 4 * m + 4
                order = list(range(4 * m, njb)) + list(range(0, 4 * m))
                ogrps = [order[i:i + 4] for i in range(0, len(order), 4)]
                accs = {tp: (ps_o.tile([VW, 512], F32, tag="pso",
                                       name=f"oe{m}_{tp}"),
                             ps_o.tile([VW, 512], F32, tag="pso",
                                       name=f"oo{m}_{tp}")) for tp in tps}
                started = {tp: False for tp in tps}
                ndone = 0
                fill = list(fillers)
                for grp in ogrps:
                    pts = {}
                    for jb in grp:
                        for tp in tps:
                            pts[(tp, jb)] = _attn_slot(m, tp, jb)
                    ndone += len(grp)
                    for tp in tps:
                        for jb in sorted(grp):
                            last = (ndone == njb and jb == max(grp))
                            _attn_pv(m, tp, jb, pts[(tp, jb)], accs[tp],
                                     start=(not started[tp]), stop=last)
                            started[tp] = True
                    if fill:
                        fill.pop(0)()
                for tp in tps:
                    _attn_drain(m, tp, accs[tp], tps[0])

            def finish_half(m, hp):
                # normalize heads 4hp..4hp+3, stage to DRAM, AllGather the
                # half-chunk so the exchange overlaps the other pair's attn
                sums4 = sums_map[(m, hp)]
                rec4 = nrmp.tile([4, 512], BF16, tag="rec4",
                                 name=f"rec4_{m}_{hp}")
                with nc.allow_low_precision(reason="bf16 softmax denom ok"):
                    nc.vector.reciprocal(rec4[:], sums4[:])
                oT2 = [otp.tile([128, 512], BF16, tag="oT",
                                name=f"oT{m}_{2 * hp + j}") for j in range(2)]
                for lh in range(4):
                    h = 4 * hp + lh
                    jt, po = lh // 2, DK * (lh % 2)
                    stage = stgp.tile([1, 512], BF16, tag="stage")
                    nc.sync.dma_start(out=stage[:], in_=rec4[lh:lh + 1, :])
                    rep = repp.tile([64, 512], BF16, tag="rep")
                    nc.gpsimd.partition_broadcast(rep[:], stage[:])
                    nc.vector.tensor_mul(oT2[jt][po:po + DK, :],
                                         osb_map[(m, h)][0:DK, :], rep[:])
                r0 = 512 * m + 256 * hp
                for j in range(2):
                    e1 = nc.gpsimd if j == 0 else nc.sync
                    e1.dma_start(
                        out=og_in[r0 + 128 * j:r0 + 128 * (j + 1), :],
                        in_=oT2[j][:])
                nc.gpsimd.collective_compute(
                    "AllGather", mybir.AluOpType.bypass,
                    replica_groups=groups,
                    ins=[og_in[r0:r0 + 256, :].opt()],
                    outs=[og_out[1024 * m + 512 * hp:
                                 1024 * m + 512 * hp + 512, :].opt()])

            # SPMD program is shared across cores, so the projection reads all
            # 8 gathered k-tiles from og_out in global head order. Head pair
            # kt (heads 2kt..2kt+1) lives at: rank = kt//4 (0=even core),
            # hp = (kt % 4) // 2, j = kt % 2.
            def o_fetch_all(m):
                o8 = [oap.tile([128, 512], BF16, tag="oa", name=f"oa{m}_{t}")
                      for t in range(DT)]
                qs = [nc.gpsimd, nc.sync]
                for kt in range(DT):
                    rank, hp, j = kt // 4, (kt % 4) // 2, kt % 2
                    row = 1024 * m + 512 * hp + 256 * rank + 128 * j
                    qs[kt % 2].dma_start(out=o8[kt][:],
                                          in_=og_out[row:row + 128, :])
                oPeer[m] = o8

            def proj_piece2(m, r2):
                r0 = 512 * m + 128 * r2
                ych = ychp.tile([128, 512], BF16, tag="ych")
                yp = ps_b.tile([128, 1024], F32, tag="psb", name="yp")[:, 0:512]
                for kt in range(DT):
                    nc.tensor.matmul(
                        yp[:],
                        lhsT=oPeer[m][kt][:, 128 * r2:128 * (r2 + 1)],
                        rhs=woT_sb[kt][:],
                        start=(kt == 0), stop=(kt == DT - 1))
                nc.vector.tensor_copy(ych[:], yp[:])
                nc.sync.dma_start(out=y[r0:r0 + 128, :], in_=ych[:])

            def qk_group(units):
                # software-pipeline proj/swap phases across a group of units
                # so the PE never waits on a single PSUM buf's evacuation
                qk_proj(*units[0])
                for i in range(1, len(units)):
                    qk_proj(*units[i])
                    qk_swap(*units[i - 1])
                qk_swap(*units[-1])

            qk_group([(0, "q", 0), (0, "k", 0), (0, "q", 1), (0, "k", 1)])
            for st in range(4):
                v_unit(st)
            attn_pair(0, (0, 1))
            finish_half(0, 0)
            qk_group([(0, "q", 2), (0, "k", 2), (0, "q", 3), (0, "k", 3)])
            attn_pair(0, (2, 3))
            finish_half(0, 1)
            for st in range(4, 8):
                v_unit(st)
            attn_pair(1, (0, 1))
            finish_half(1, 0)
            for st in range(8, 12):
                v_unit(st)
            attn_pair(1, (2, 3))
            finish_half(1, 1)
            for st in range(12, 16):
                v_unit(st)
            qk_group([(1, "q", 0), (1, "k", 0), (1, "q", 1), (1, "k", 1)])
            o_fetch_all(0)
            proj_piece2(0, 0)
            proj_piece2(0, 1)
            proj_piece2(0, 2)
            proj_piece2(0, 3)
            qk_group([(1, "q", 2), (1, "k", 2), (1, "q", 3), (1, "k", 3)])
            attn_pair(2, (0, 1))
            finish_half(2, 0)
            o_fetch_all(1)
            proj_piece2(1, 0)
            proj_piece2(1, 1)
            proj_piece2(1, 2)
            proj_piece2(1, 3)
            attn_pair(2, (2, 3))
            finish_half(2, 1)
            attn_pair(3, (0, 1))
            finish_half(3, 0)
            o_fetch_all(2)
            proj_piece2(2, 0)
            proj_piece2(2, 1)
            proj_piece2(2, 2)
            proj_piece2(2, 3)
            attn_pair(3, (2, 3))
            finish_half(3, 1)
            # keep the PE clock warm across the final AllGather wait
            ps_d = ps_b.tile([128, 1024], F32, tag="psb", name="ps_dummy")
            for _ in range(36):
                nc.tensor.matmul(ps_d[:, 0:512], lhsT=swap_sb[:],
                                 rhs=cos_sb[:, 0:512], start=True, stop=True)
            o_fetch_all(3)
            for r2 in range(4):
                proj_piece2(3, r2)

    nc.compile()
    return nc


def _prep_inputs(x, Wq, Wk, Wv, Wo, cos_emb, sin_emb, token_positions):
    bf = ml_dtypes.bfloat16
    cos_g = np.asarray(cos_emb)[np.asarray(token_positions)]  # [S, DK]
    sin_g = np.asarray(sin_emb)[np.asarray(token_positions)]
    # [128, S]: partition p -> head-dim p % 64
    cosT = np.ascontiguousarray(np.tile(cos_g.T, (2, 1))).astype(bf)
    sinT = np.ascontiguousarray(np.tile(sin_g.T, (2, 1))).astype(bf)
    # rotate-half-interleaved as a matmul: rh = SWAP @ q (per 128-dim tile)
    swap = np.zeros((128, 128), np.float32)
    for j in range(64):
        swap[2 * j, 2 * j + 1] = -1.0
        swap[2 * j + 1, 2 * j] = 1.0
    swapT = np.ascontiguousarray(swap.T).astype(bf)
    # causal 0/1 mask for the diagonal 128x128 block in S^T=[j,i] layout,
    # applied multiplicatively AFTER the exp
    jj = np.arange(128)[:, None]
    ii = np.arange(128)[None, :]
    maskT = np.where(ii >= jj, 1.0, 0.0).astype(bf)

    in_maps = []
    for c in range(NCORES):
        b, hh = c // 2, c % 2
        cols = slice(512 * hh, 512 * (hh + 1))   # my heads' dims
        ocols = slice(512 * hh, 512 * (hh + 1))  # my output columns
        in_maps.append({
            "xT": np.ascontiguousarray(np.asarray(x)[b].T).astype(bf),
            "wqT": np.ascontiguousarray(np.asarray(Wq)[cols, :].T).astype(bf),
            "wkT": np.ascontiguousarray(np.asarray(Wk)[cols, :].T).astype(bf),
            "wvT": np.ascontiguousarray(np.asarray(Wv)[cols, :].T).astype(bf),
            "woT": np.ascontiguousarray(np.asarray(Wo)[ocols, :].T).astype(bf),
            "cosT": cosT, "sinT": sinT, "swapT": swapT, "maskT": maskT,
        })
    return in_maps


def kernel(x, Wq, Wk, Wv, Wo, cos_emb, sin_emb, token_positions, **run_kwargs):
    if "nc" not in _compiled:
        _compiled["nc"] = _build_nc()
    nc = _compiled["nc"]
    in_maps = _prep_inputs(x, Wq, Wk, Wv, Wo, cos_emb, sin_emb, token_positions)
    res = run_bass_kernel_spmd(nc, in_maps, list(range(NCORES)), **run_kwargs)
    out = np.stack([
        np.concatenate([res.results[2 * b]["y"], res.results[2 * b + 1]["y"]],
                       axis=1)
        for b in range(B)
    ]).astype(np.float32)
    if run_kwargs:
        kernel.last_result = res
    return out


# revision 37
# speedup vs baseline: 1.1953x; 1.1953x over previous
"""Causal multi-head self-attention with RoPE on 8 NeuronCores.

Sharding: batch (4) x head-halves (2) -> 8 cores. Core c handles batch c//2,
heads [8*(c%2), 8*(c%2)+8). Pipelined schedule: QKV projection, attention
i-chunks and output projection are interleaved at tile granularity so the
Scalar engine (softmax exp) and PE (matmuls) overlap across the whole kernel.
The pair of cores sharing a batch exchanges normalized attention outputs with
a per-chunk AllGather; each core then projects its own half of the output
columns with the full 1024-dim contraction (no AllReduce, half the traffic).
"""

import numpy as np
import ml_dtypes

import concourse.bacc as bacc
import concourse.bass as bass
import concourse.mybir as mybir
from concourse.tile import TileContext
from concourse.bass_utils import run_bass_kernel_spmd

B, S, D, H = 4, 2048, 1024, 16
HL = 8          # heads per core
DK = 64         # head dim
NCORES = 8
DT = D // 128   # 8 d-tiles (contraction tiles)
OT = HL * DK // 128   # 4 o-tiles for Q^T/K^T ([128, S] each, 2 heads per tile)
ST = S // 128   # 16 s-tiles
VW = DK + 1     # V columns per head incl. ones column

BF16 = mybir.dt.bfloat16
F32 = mybir.dt.float32
NEG = -1.0e9
EXP = mybir.ActivationFunctionType.Exp

_compiled = {}


def _build_nc():
    nc = bacc.Bacc("TRN2", target_bir_lowering=False, debug=False,
                   num_devices=NCORES)

    xT = nc.dram_tensor("xT", [D, S], BF16, kind="ExternalInput")
    wqT = nc.dram_tensor("wqT", [D, HL * DK], BF16, kind="ExternalInput")
    wkT = nc.dram_tensor("wkT", [D, HL * DK], BF16, kind="ExternalInput")
    wvT = nc.dram_tensor("wvT", [D, HL * DK], BF16, kind="ExternalInput")
    woT = nc.dram_tensor("woT", [D, D // 2], BF16, kind="ExternalInput")
    cosT = nc.dram_tensor("cosT", [128, S], BF16, kind="ExternalInput")
    sinT = nc.dram_tensor("sinT", [128, S], BF16, kind="ExternalInput")
    swapT = nc.dram_tensor("swapT", [128, 128], BF16, kind="ExternalInput")
    maskT = nc.dram_tensor("maskT", [128, 128], BF16, kind="ExternalInput")
    y = nc.dram_tensor("y", [S, D // 2], BF16, kind="ExternalOutput")

    groups = [[0, 1], [2, 3], [4, 5], [6, 7]]

    with TileContext(nc) as tc:
        with (
            tc.tile_pool(name="big", bufs=1) as big,
            tc.tile_pool(name="rawp", bufs=2) as rawp,
            tc.tile_pool(name="t1p", bufs=2) as t1p,
            tc.tile_pool(name="t2p", bufs=2) as t2p,
            tc.tile_pool(name="ptp", bufs=12) as ptp,
            tc.tile_pool(name="osbp", bufs=10) as osbp,
            tc.tile_pool(name="otp", bufs=8) as otp,
            tc.tile_pool(name="oap", bufs=8) as oap,
            tc.tile_pool(name="ychp", bufs=3) as ychp,
            tc.tile_pool(name="nrmp", bufs=2) as nrmp,
            tc.tile_pool(name="stgp", bufs=3) as stgp,
            tc.tile_pool(name="repp", bufs=3) as repp,
            tc.tile_pool(name="ps_b", bufs=2, space="PSUM") as ps_b,
            tc.tile_pool(name="ps_o", bufs=4, space="PSUM") as ps_o,
            tc.tile_pool(name="dram", bufs=1, space="DRAM") as dram,
        ):
            # ---- input DMAs, split across queues so QKV can start early ----
            w_sb = {"q": [None] * DT, "k": [None] * DT, "v": [None] * DT}
            xT_sb = [None] * DT
            for k in range(DT):
                e1, e2 = (nc.sync, nc.scalar) if k % 2 == 0 else (nc.scalar, nc.sync)
                xt = big.tile([128, S], BF16, tag=f"xT{k}", name=f"xT{k}")
                e1.dma_start(out=xt[:, 0:1024], in_=xT[128 * k:128 * (k + 1),
                                                       0:1024])
                xT_sb[k] = xt
                wt = big.tile([128, HL * DK], BF16, tag=f"wq{k}", name=f"wq{k}")
                e2.dma_start(out=wt[:], in_=wqT[128 * k:128 * (k + 1), :])
                w_sb["q"][k] = wt
            cos_sb = big.tile([128, S], BF16, tag="cos")
            nc.scalar.dma_start(out=cos_sb[:], in_=cosT[:])
            sin_sb = big.tile([128, S], BF16, tag="sin")
            nc.scalar.dma_start(out=sin_sb[:], in_=sinT[:])
            swap_sb = big.tile([128, 128], BF16, tag="swap")
            nc.scalar.dma_start(out=swap_sb[:], in_=swapT[:])
            for k in range(DT):
                e1 = nc.sync if k % 2 == 0 else nc.scalar
                wt = big.tile([128, HL * DK], BF16, tag=f"wk{k}", name=f"wk{k}")
                e1.dma_start(out=wt[:], in_=wkT[128 * k:128 * (k + 1), :])
                w_sb["k"][k] = wt
            mask_sb = big.tile([128, 128], BF16, tag="mask")
            nc.scalar.dma_start(out=mask_sb[:], in_=maskT[:])
            for k in range(DT):
                e1 = nc.sync if k % 2 == 0 else nc.scalar
                e1.dma_start(out=xT_sb[k][:, 1024:2048],
                             in_=xT[128 * k:128 * (k + 1), 1024:2048])
            for k in range(DT):
                e1 = nc.sync if k % 2 == 0 else nc.scalar
                wt = big.tile([128, HL * DK], BF16, tag=f"wv{k}", name=f"wv{k}")
                e1.dma_start(out=wt[:], in_=wvT[128 * k:128 * (k + 1), :])
                w_sb["v"][k] = wt
            woT_sb = []
            for k in range(DT):
                e1 = nc.sync if k % 2 == 0 else nc.scalar
                wt = big.tile([128, D // 2], BF16, tag=f"wo{k}", name=f"wo{k}")
                e1.dma_start(out=wt[:], in_=woT[128 * k:128 * (k + 1), :])
                woT_sb.append(wt)

            vaug = []
            for st in range(ST):
                vt = big.tile([128, HL * VW], BF16, tag=f"va{st}", name=f"va{st}")
                ones = vt[:].rearrange("p (h d) -> p h d", d=VW)[:, :, DK:VW]
                nc.gpsimd.memset(ones, 1.0)
                vaug.append(vt)
            qrot = [big.tile([128, S], BF16, tag=f"qr{t}", name=f"qr{t}")
                    for t in range(OT)]
            krot = [big.tile([128, S], BF16, tag=f"kr{t}", name=f"kr{t}")
                    for t in range(OT)]

            oT = {}        # m -> [OT tiles [128,512]] my normalized o chunk
            oPeer = {}     # m -> [OT tiles [128,512]] peer half via AllGather
            osb_map = {}   # (m, h) -> osb tile
            sums_map = {}  # m -> sums8 tile
            # o exchange buffers, half-chunk granular: half hp of chunk m is
            # og_in rows [512m+256hp, +256) = heads 4hp..4hp+3 of this core,
            # cols = 512 i's of chunk m. og_out block [1024m+512hp, +512) =
            # [rank0's 256 rows; rank1's 256 rows].
            og_in = dram.tile([4 * 512, 512], BF16)
            og_out = dram.tile([4 * 1024, 512], BF16)

            qk_state = {}

            def qk_proj(h, qk, t):
                c0 = 1024 * h
                ps = ps_b.tile([128, 1024], F32, tag="psb", name="ps")
                for k in range(DT):
                    for cc in range(2):
                        nc.tensor.matmul(
                            ps[:, 512 * cc:512 * (cc + 1)],
                            lhsT=w_sb[qk][k][:, 128 * t:128 * (t + 1)],
                            rhs=xT_sb[k][:, c0 + 512 * cc:c0 + 512 * (cc + 1)],
                            start=(k == 0), stop=(k == DT - 1))
                raw = rawp.tile([128, 1024], BF16, tag="raw")
                nc.scalar.copy(out=raw[:], in_=ps[:])
                t1 = t1p.tile([128, 1024], BF16, tag="t1")
                nc.vector.tensor_mul(t1[:], raw[:], cos_sb[:, c0:c0 + 1024])
                qk_state[(h, qk, t)] = (raw, t1)

            def qk_swap(h, qk, t):
                c0 = 1024 * h
                raw, t1 = qk_state.pop((h, qk, t))
                ps2 = ps_b.tile([128, 1024], F32, tag="psb", name="ps2")
                for cc in range(2):
                    nc.tensor.matmul(
                        ps2[:, 512 * cc:512 * (cc + 1)], lhsT=swap_sb[:],
                        rhs=raw[:, 512 * cc:512 * (cc + 1)],
                        start=True, stop=True)
                t2 = t2p.tile([128, 1024], BF16, tag="t2")
                nc.vector.tensor_mul(t2[:], ps2[:], sin_sb[:, c0:c0 + 1024])
                dst = qrot[t] if qk == "q" else krot[t]
                nc.vector.tensor_add(dst[:, c0:c0 + 1024], t1[:], t2[:])

            def qk_unit(h, qk, t):
                qk_proj(h, qk, t)
                qk_swap(h, qk, t)

            def v_unit(st):
                ps = ps_b.tile([128, 1024], F32, tag="psb", name="psv")[:, 0:512]
                for k in range(DT):
                    nc.tensor.matmul(
                        ps[:], lhsT=xT_sb[k][:, 128 * st:128 * (st + 1)],
                        rhs=w_sb["v"][k][:], start=(k == 0), stop=(k == DT - 1))
                dst = vaug[st][:].rearrange("p (h d) -> p h d", d=VW)[:, :, 0:DK]
                src = ps[:].rearrange("p (h d) -> p h d", d=DK)
                nc.vector.tensor_copy(dst, src)

            def _attn_slot(m, tp, jb):
                # QK^T (row-tiled head pair) -> exp -> post-exp 0/1 mask
                i0 = 512 * m
                j0 = 128 * jb
                dlt = max(0, j0 - i0)
                s_ps = ps_b.tile([128, 1024], F32, tag="psb", name="sps")
                for half, po in ((0, 0), (1, DK)):
                    nc.tensor.matmul(
                        s_ps[:, 512 * half + dlt:512 * (half + 1)],
                        lhsT=krot[tp][po:po + DK, j0:j0 + 128],
                        rhs=qrot[tp][po:po + DK, i0 + dlt:i0 + 512],
                        start=True, stop=True)
                pT = ptp.tile([128, 1024], BF16, tag="pT")
                pT3 = pT[:].rearrange("p (b f) -> p b f", b=2)
                nc.scalar.activation(
                    pT3[:, :, dlt:512],
                    s_ps[:].rearrange("p (b f) -> p b f", b=2)[:, :, dlt:512],
                    EXP, scale=0.125)
                if j0 >= i0:
                    nc.vector.tensor_mul(
                        pT3[:, :, dlt:dlt + 128],
                        pT3[:, :, dlt:dlt + 128],
                        mask_sb[:].rearrange("p (b f) -> p b f", b=1)
                        .broadcast_to([128, 2, 128]))
                return pT

            def _attn_pv(m, tp, jb, pT, accs, start, stop):
                dlt = max(0, 128 * jb - 512 * m)
                for half in (0, 1):
                    nc.tensor.matmul(
                        accs[half][:, dlt:512],
                        lhsT=vaug[jb][:, VW * (2 * tp + half):
                                      VW * (2 * tp + half + 1)],
                        rhs=pT[:, 512 * half + dlt:512 * (half + 1)],
                        start=start, stop=stop,
                    )

            def _attn_drain(m, tp, accs, pair_base):
                osb_e = osbp.tile([VW, 512], BF16, tag="osb", name="osbe")
                nc.vector.tensor_copy(osb_e[:], accs[0][:])
                osb_o = osbp.tile([VW, 512], BF16, tag="osb", name="osbo")
                nc.vector.tensor_copy(osb_o[:], accs[1][:])
                hp = pair_base // 2
                if (m, hp) not in sums_map:
                    sums_map[(m, hp)] = nrmp.tile([4, 512], BF16, tag="sums4",
                                                  name=f"sums4_{m}_{hp}")
                sums4 = sums_map[(m, hp)]
                r = 2 * (tp - pair_base)
                nc.sync.dma_start(out=sums4[r:r + 1, :], in_=osb_e[DK:VW, :])
                nc.sync.dma_start(out=sums4[r + 1:r + 2, :],
                                  in_=osb_o[DK:VW, :])
                osb_map[(m, 2 * tp)] = osb_e
                osb_map[(m, 2 * tp + 1)] = osb_o

            def attn_pair(m, tps, fillers=()):
                # two head-pair tiles interleaved slot-by-slot so the PE queue
                # always has ready work; PV in groups of 4 j-blocks per tp
                i0 = 512 * m
                njb = 4 * m + 4
                order = list(range(4 * m, njb)) + list(range(0, 4 * m))
                ogrps = [order[i:i + 4] for i in range(0, len(order), 4)]
                accs = {tp: (ps_o.tile([VW, 512], F32, tag="pso",
                                       name=f"oe{m}_{tp}"),
                             ps_o.tile([VW, 512], F32, tag="pso",
                                       name=f"oo{m}_{tp}")) for tp in tps}
                started = {tp: False for tp in tps}
                ndone = 0
                fill = list(fillers)
                for grp in ogrps:
                    pts = {}
                    for jb in grp:
                        for tp in tps:
                            pts[(tp, jb)] = _attn_slot(m, tp, jb)
                    ndone += len(grp)
                    for tp in tps:
                        for jb in sorted(grp):
                            last = (ndone == njb and jb == max(grp))
                            _attn_pv(m, tp, jb, pts[(tp, jb)], accs[tp],
                                     start=(not started[tp]), stop=last)
                            started[tp] = True
                    if fill:
                        fill.pop(0)()
                for tp in tps:
                    _attn_drain(m, tp, accs[tp], tps[0])

            def finish_half(m, hp):
                # normalize heads 4hp..4hp+3, stage to DRAM, AllGather the
                # half-chunk so the exchange overlaps the other pair's attn
                sums4 = sums_map[(m, hp)]
                rec4 = nrmp.tile([4, 512], BF16, tag="rec4",
                                 name=f"rec4_{m}_{hp}")
                with nc.allow_low_precision(reason="bf16 softmax denom ok"):
                    nc.vector.reciprocal(rec4[:], sums4[:])
                oT2 = [otp.tile([128, 512], BF16, tag="oT",
                                name=f"oT{m}_{2 * hp + j}") for j in range(2)]
                for lh in range(4):
                    h = 4 * hp + lh
                    jt, po = lh // 2, DK * (lh % 2)
                    stage = stgp.tile([1, 512], BF16, tag="stage")
                    nc.sync.dma_start(out=stage[:], in_=rec4[lh:lh + 1, :])
                    rep = repp.tile([64, 512], BF16, tag="rep")
                    nc.gpsimd.partition_broadcast(rep[:], stage[:])
                    nc.vector.tensor_mul(oT2[jt][po:po + DK, :],
                                         osb_map[(m, h)][0:DK, :], rep[:])
                r0 = 512 * m + 256 * hp
                for j in range(2):
                    e1 = nc.gpsimd if j == 0 else nc.sync
                    e1.dma_start(
                        out=og_in[r0 + 128 * j:r0 + 128 * (j + 1), :],
                        in_=oT2[j][:])
                nc.gpsimd.collective_compute(
                    "AllGather", mybir.AluOpType.bypass,
                    replica_groups=groups,
                    ins=[og_in[r0:r0 + 256, :].opt()],
                    outs=[og_out[1024 * m + 512 * hp:
                                 1024 * m + 512 * hp + 512, :].opt()])

            # SPMD program is shared across cores, so the projection reads all
            # 8 gathered k-tiles from og_out in global head order. Head pair
            # kt (heads 2kt..2kt+1) lives at: rank = kt//4 (0=even core),
            # hp = (kt % 4) // 2, j = kt % 2.
            def o_fetch_all(m):
                o8 = [oap.tile([128, 512], BF16, tag="oa", name=f"oa{m}_{t}")
                      for t in range(DT)]
                qs = [nc.gpsimd, nc.sync]
                for kt in range(DT):
                    rank, hp, j = kt // 4, (kt % 4) // 2, kt % 2
                    row = 1024 * m + 512 * hp + 256 * rank + 128 * j
                    qs[kt % 2].dma_start(out=o8[kt][:],
                                          in_=og_out[row:row + 128, :])
                oPeer[m] = o8

            def proj_piece2(m, r2):
                r0 = 512 * m + 128 * r2
                ych = ychp.tile([128, 512], BF16, tag="ych")
                yp = ps_b.tile([128, 1024], F32, tag="psb", name="yp")[:, 0:512]
                for kt in range(DT):
                    nc.tensor.matmul(
                        yp[:],
                        lhsT=oPeer[m][kt][:, 128 * r2:128 * (r2 + 1)],
                        rhs=woT_sb[kt][:],
                        start=(kt == 0), stop=(kt == DT - 1))
                nc.vector.tensor_copy(ych[:], yp[:])
                nc.sync.dma_start(out=y[r0:r0 + 128, :], in_=ych[:])

            def qk_group(units):
                # software-pipeline proj/swap phases across a group of units
                # so the PE never waits on a single PSUM buf's evacuation
                qk_proj(*units[0])
                for i in range(1, len(units)):
                    qk_proj(*units[i])
                    qk_swap(*units[i - 1])
                qk_swap(*units[-1])

            qk_group([(0, "q", 0), (0, "k", 0), (0, "q", 1), (0, "k", 1)])
            for st in range(4):
                v_unit(st)
            attn_pair(0, (0, 1))
            finish_half(0, 0)
            qk_group([(0, "q", 2), (0, "k", 2), (0, "q", 3), (0, "k", 3)])
            attn_pair(0, (2, 3))
            finish_half(0, 1)
            for st in range(4, 8):
                v_unit(st)
            attn_pair(1, (0, 1))
            finish_half(1, 0)
            for st in range(8, 12):
                v_unit(st)
            attn_pair(1, (2, 3))
            finish_half(1, 1)
            for st in range(12, 16):
                v_unit(st)
            qk_group([(1, "q", 0), (1, "k", 0), (1, "q", 1), (1, "k", 1)])
            o_fetch_all(0)
            proj_piece2(0, 0)
            proj_piece2(0, 1)
            proj_piece2(0, 2)
            proj_piece2(0, 3)
            qk_group([(1, "q", 2), (1, "k", 2), (1, "q", 3), (1, "k", 3)])
            attn_pair(2, (0, 1))
            finish_half(2, 0)
            o_fetch_all(1)
            proj_piece2(1, 0)
            proj_piece2(1, 1)
            proj_piece2(1, 2)
            proj_piece2(1, 3)
            attn_pair(2, (2, 3))
            finish_half(2, 1)
            attn_pair(3, (0, 1))
            finish_half(3, 0)
            o_fetch_all(2)
            proj_piece2(2, 0)
            proj_piece2(2, 1)
            proj_piece2(2, 2)
            proj_piece2(2, 3)
            attn_pair(3, (2, 3))
            finish_half(3, 1)
            # keep the PE clock warm across the final AllGather wait
            ps_d = ps_b.tile([128, 1024], F32, tag="psb", name="ps_dummy")
            for _ in range(36):
                nc.tensor.matmul(ps_d[:, 0:512], lhsT=swap_sb[:],
                                 rhs=cos_sb[:, 0:512], start=True, stop=True)
            o_fetch_all(3)
            for r2 in range(4):
                proj_piece2(3, r2)

    nc.compile()
    return nc


def _prep_inputs(x, Wq, Wk, Wv, Wo, cos_emb, sin_emb, token_positions):
    bf = ml_dtypes.bfloat16
    cos_g = np.asarray(cos_emb)[np.asarray(token_positions)]  # [S, DK]
    sin_g = np.asarray(sin_emb)[np.asarray(token_positions)]
    # [128, S]: partition p -> head-dim p % 64
    cosT = np.ascontiguousarray(np.tile(cos_g.T, (2, 1))).astype(bf)
    sinT = np.ascontiguousarray(np.tile(sin_g.T, (2, 1))).astype(bf)
    # rotate-half-interleaved as a matmul: rh = SWAP @ q (per 128-dim tile)
    swap = np.zeros((128, 128), np.float32)
    for j in range(64):
        swap[2 * j, 2 * j + 1] = -1.0
        swap[2 * j + 1, 2 * j] = 1.0
    swapT = np.ascontiguousarray(swap.T).astype(bf)
    # causal 0/1 mask for the diagonal 128x128 block in S^T=[j,i] layout,
    # applied multiplicatively AFTER the exp
    jj = np.arange(128)[:, None]
    ii = np.arange(128)[None, :]
    maskT = np.where(ii >= jj, 1.0, 0.0).astype(bf)

    in_maps = []
    for c in range(NCORES):
        b, hh = c // 2, c % 2
        cols = slice(512 * hh, 512 * (hh + 1))   # my heads' dims
        ocols = slice(512 * hh, 512 * (hh + 1))  # my output columns
        in_maps.append({
            "xT": np.ascontiguousarray(np.asarray(x)[b].T).astype(bf),
            "wqT": np.ascontiguousarray(np.asarray(Wq)[cols, :].T).astype(bf),
            "wkT": np.ascontiguousarray(np.asarray(Wk)[cols, :].T).astype(bf),
            "wvT": np.ascontiguousarray(np.asarray(Wv)[cols, :].T).astype(bf),
            "woT": np.ascontiguousarray(np.asarray(Wo)[ocols, :].T).astype(bf),
            "cosT": cosT, "sinT": sinT, "swapT": swapT, "maskT": maskT,
        })
    return in_maps


def kernel(x, Wq, Wk, Wv, Wo, cos_emb, sin_emb, token_positions, **run_kwargs):
    if "nc" not in _compiled:
        _compiled["nc"] = _build_nc()
    nc = _compiled["nc"]
    in_maps = _prep_inputs(x, Wq, Wk, Wv, Wo, cos_emb, sin_emb, token_positions)
    res = run_bass_kernel_spmd(nc, in_maps, list(range(NCORES)), **run_kwargs)
    out = np.stack([
        np.concatenate([res.results[2 * b]["y"], res.results[2 * b + 1]["y"]],
                       axis=1)
        for b in range(B)
    ]).astype(np.float32)
    if run_kwargs:
        kernel.last_result = res
    return out
